# revision 48
# baseline (speedup 1.0000x reference)
"""Trainium2 Bass kernel: aspect-level sentiment classification head.

  aspect[b] = mean(last_hidden_state[b, start_b:end_b, :])   (ragged spans)
  out = concat([pooled, aspect], -1) @ W.T + b

Strategy: data-parallel over batch (8 samples per core, 8 cores).  Only the
span rows of last_hidden_state are ever needed, so each core *gathers* just
those rows from DRAM with an indirect DMA whose row indices are computed
on-device from position_indices.  Spans are padded to L = 32*m rows (m =
power of two chosen from the max span length at call time); rows past the
span end get an out-of-bounds index and are skipped by the DMA bounds check,
landing as zeros in the pre-zeroed SBUF tile.  A constant block-ones matmul
sums the gathered rows per sample, a per-partition 1/len scale turns sums
into means, and PE transposes feed an accumulated 12-chunk GEMM against
host-pre-transposed W.
"""

import os
import sys

if "/opt/trn_rl_repo" not in sys.path:
    sys.path.insert(0, "/opt/trn_rl_repo")

import numpy as np

import concourse.bass as bass
import concourse.tile as tile
from concourse import bacc, mybir
from concourse.bass import IndirectOffsetOnAxis
from concourse.bass_utils import run_bass_kernel_spmd
from concourse.masks import make_identity

F32 = mybir.dt.float32
I32 = mybir.dt.int32

B, S, H, C = 64, 4096, 768, 3
NCORES = 8
BL = B // NCORES          # samples per core
P = 128
HC = H // P               # 6 hidden chunks of 128
KC = 2 * H // P           # 12 contraction chunks in the final GEMM
# Marker added to row indices outside the span so the DMA bounds check
# skips them.  Must exceed bounds_check (BL*S) but keep idx * H < 2^31
# (the descriptor address math is 32-bit).
OOB = 1_000_000


def _log2(x: int) -> int:
    l = x.bit_length() - 1
    assert 1 << l == x
    return l


def build(m: int):
    """Build + compile the per-core SPMD program for spans up to 32*m rows."""
    assert m & (m - 1) == 0 and 1 <= m <= S // 32
    L = 32 * m               # padded rows per sample
    nblk = BL * m            # 32-row blocks per core
    G = nblk // 4            # gather groups of 128 rows
    cols = max(1, 4 // m)    # samples covered by one group
    gps = max(1, m // 4)     # groups per sample
    lm = _log2(m)

    nc = bacc.Bacc("TRN2", target_bir_lowering=False, debug=False,
                   num_devices=NCORES)
    lhs = nc.dram_tensor("lhs", [BL * S, H], F32, kind="ExternalInput").ap()
    pooled_r = nc.dram_tensor("pooled_r", [HC * BL, P], F32,
                              kind="ExternalInput").ap()
    w_r = nc.dram_tensor("w_r", [KC * C, P], F32, kind="ExternalInput").ap()
    pos = nc.dram_tensor("pos", [BL, 2], I32, kind="ExternalInput").ap()
    bias = nc.dram_tensor("bias", [BL, C], F32, kind="ExternalInput").ap()
    out = nc.dram_tensor("out", [BL, C], F32, kind="ExternalOutput").ap()

    with tile.TileContext(nc) as tc:
        with (
            tc.tile_pool(name="const", bufs=1) as cp,
            tc.tile_pool(name="rows", bufs=4) as rp,
            tc.tile_pool(name="pmisc", bufs=2, space="PSUM") as pm,
            tc.tile_pool(name="pasp", bufs=1, space="PSUM") as pa,
        ):
            # ---- params ------------------------------------------------
            pos_i = cp.tile([BL, 2], I32, tag="pos_i")
            nc.sync.dma_start(pos_i[:], pos[:, :])
            pooled_sb = cp.tile([HC * BL, P], F32, tag="pooled_sb")
            nc.sync.dma_start(pooled_sb[:], pooled_r[:, :])
            w_sb = cp.tile([KC * C, P], F32, tag="w_sb")
            nc.sync.dma_start(w_sb[:], w_r[:, :])
            bias_sb = cp.tile([BL, C], F32, tag="bias_sb")
            nc.sync.dma_start(bias_sb[:], bias[:, :])

            # ---- constants ---------------------------------------------
            id48 = cp.tile([HC * BL, HC * BL], F32, tag="id48")
            make_identity(nc, id48[:])
            idb = cp.tile([BL, BL], F32, tag="idb")
            make_identity(nc, idb[:])

            # ones_all[p, g*8+s] = 1 iff s == (4g + p//32) >> lm, i.e. row p
            # of gather group g belongs to sample s.  Built once for all
            # groups with two affine selects over the (g, s) pattern dims.
            ones_all = cp.tile([P, 8 * G], F32, tag="ones_all")
            nc.gpsimd.memset(ones_all[:], 1.0)
            # keep iff p - 32m*s + 128g >= 0
            nc.gpsimd.affine_select(
                out=ones_all[:], in_=ones_all[:],
                compare_op=mybir.AluOpType.is_ge, fill=0.0, base=0,
                channel_multiplier=1, pattern=[[128, G], [-32 * m, BL]])
            # keep iff p - 32m*s + 128g <= 32m-1  (negated for is_ge)
            nc.gpsimd.affine_select(
                out=ones_all[:], in_=ones_all[:],
                compare_op=mybir.AluOpType.is_ge, fill=0.0,
                base=32 * m - 1, channel_multiplier=-1,
                pattern=[[-128, G], [32 * m, BL]])

            # ---- per-sample gather indices [BL, L] ----------------------
            # All index math in f32 (tensor_scalar AP operands must be f32);
            # every value stays < 2^24 so the arithmetic is exact.
            pos_f = cp.tile([BL, 2], F32, tag="pos_f")
            nc.vector.tensor_copy(pos_f[:], pos_i[:])
            iota_j = cp.tile([BL, L], F32, tag="iota_j")
            nc.gpsimd.iota(iota_j[:], pattern=[[1, L]], base=0,
                           channel_multiplier=0,
                           allow_small_or_imprecise_dtypes=True)
            base_f = cp.tile([BL, 1], F32, tag="base_f")
            nc.gpsimd.iota(base_f[:], pattern=[[1, 1]], base=0,
                           channel_multiplier=S,
                           allow_small_or_imprecise_dtypes=True)

            # row = start + j ; rows with row >= end get +OOB (skipped)
            row_f = cp.tile([BL, L], F32, tag="row_f")
            nc.vector.tensor_scalar(row_f[:], iota_j[:], pos_f[:, 0:1], None,
                                    mybir.AluOpType.add)
            oob_f = cp.tile([BL, L], F32, tag="oob_f")
            nc.vector.tensor_scalar(oob_f[:], row_f[:], pos_f[:, 1:2],
                                    float(OOB), mybir.AluOpType.is_ge,
                                    mybir.AluOpType.mult)
            rowo_f = cp.tile([BL, L], F32, tag="rowo_f")
            nc.vector.tensor_tensor(out=rowo_f[:], in0=row_f[:], in1=oob_f[:],
                                    op=mybir.AluOpType.add)
            idx8_f = cp.tile([BL, L], F32, tag="idx8_f")
            nc.vector.tensor_scalar(idx8_f[:], rowo_f[:], base_f[:, 0:1],
                                    None, mybir.AluOpType.add)
            idx8 = cp.tile([BL, L], I32, tag="idx8")
            nc.vector.tensor_copy(idx8[:], idx8_f[:])

            # 1 / span length per sample
            len_f = cp.tile([BL, 1], F32, tag="len_f")
            nc.vector.tensor_tensor(out=len_f[:], in0=pos_f[:, 1:2],
                                    in1=pos_f[:, 0:1],
                                    op=mybir.AluOpType.subtract)
            recip = cp.tile([BL, 1], F32, tag="recip")
            nc.vector.reciprocal(recip[:], len_f[:])

            # ---- transpose params via PE -------------------------------
            pT_ps = pm.tile([P, HC * BL], F32, tag="pmisc", name="pT_ps")
            nc.tensor.transpose(pT_ps[:], pooled_sb[:], id48[:])
            pT = cp.tile([P, HC * BL], F32, tag="pT")
            nc.vector.tensor_copy(pT[:], pT_ps[:])

            wT_ps = pm.tile([P, KC * C], F32, tag="pmisc", name="wT_ps")
            nc.tensor.transpose(wT_ps[:], w_sb[:], id48[: KC * C, : KC * C])
            wT = cp.tile([P, KC * C], F32, tag="wT")
            nc.vector.tensor_copy(wT[:], wT_ps[:])

            # ---- gather + block-ones sum -------------------------------
            # psum span sums [BL, H] split into two banks (512 + 256)
            sum_a = pa.tile([BL, 512], F32, tag="sum_a")
            sum_b = pa.tile([BL, H - 512], F32, tag="sum_b")

            for g in range(G):
                rows_t = rp.tile([P, H], F32, tag="rows")
                nc.vector.memset(rows_t[:], 0.0)
                # The HW reads one offset per offset-tile *partition*, so
                # redistribute this group's [cols, 128/cols] slice of idx8
                # into partition-per-row [128, 1] layout with a local DMA.
                s_lo = (4 * g) // m
                j0 = ((4 * g) % m) * 32
                off_ap = (idx8[s_lo:s_lo + cols, j0:j0 + 128 // cols]
                          if cols > 1 else idx8[s_lo:s_lo + 1, j0:j0 + 128])
                idx128 = rp.tile([P, 1], I32, tag="idx128")
                nc.gpsimd.dma_start(out=idx128[:], in_=off_ap)
                nc.gpsimd.indirect_dma_start(
                    out=rows_t[:], out_offset=None, in_=lhs[:, :],
                    in_offset=IndirectOffsetOnAxis(ap=idx128[:, 0:1], axis=0),
                    bounds_check=BL * S - 1, oob_is_err=False)

                first = g == 0
                last = g == G - 1
                nc.tensor.matmul(out=sum_a[:, :],
                                 lhsT=ones_all[:, 8 * g:8 * g + BL],
                                 rhs=rows_t[:, :512], start=first, stop=last)
                nc.tensor.matmul(out=sum_b[:, :],
                                 lhsT=ones_all[:, 8 * g:8 * g + BL],
                                 rhs=rows_t[:, 512:], start=first, stop=last)

            # ---- mean + transpose into GEMM layout ---------------------
            asp_sb = cp.tile([BL, H], F32, tag="asp_sb")
            nc.vector.tensor_scalar(asp_sb[:, :512], sum_a[:], recip[:, 0:1],
                                    None, mybir.AluOpType.mult)
            nc.vector.tensor_scalar(asp_sb[:, 512:], sum_b[:], recip[:, 0:1],
                                    None, mybir.AluOpType.mult)

            aspT_ps = pm.tile([P, HC * BL], F32, tag="pmisc", name="aspT_ps")
            for c in range(HC):
                nc.tensor.transpose(aspT_ps[:, c * BL:(c + 1) * BL],
                                    asp_sb[:, c * P:(c + 1) * P], idb[:])
            aspT_sb = cp.tile([P, HC * BL], F32, tag="aspT_sb")
            nc.vector.tensor_copy(aspT_sb[:], aspT_ps[:])

            # ---- final GEMM out[b, j] = sum_k featT[k, b] * wT[k, j] ----
            out_ps = pm.tile([BL, C], F32, tag="pmisc", name="out_ps")
            for c in range(KC):
                featT = (pT[:, (c * BL):(c + 1) * BL] if c < HC
                         else aspT_sb[:, (c - HC) * BL:(c - HC + 1) * BL])
                nc.tensor.matmul(out=out_ps[:], lhsT=featT,
                                 rhs=wT[:, c * C:(c + 1) * C],
                                 start=(c == 0), stop=(c == KC - 1))

            out_sb = cp.tile([BL, C], F32, tag="out_sb")
            nc.vector.tensor_add(out_sb[:], out_ps[:], bias_sb[:])
            nc.sync.dma_start(out[:, :], out_sb[:])

    nc.compile()
    return nc


def build_raw(m: int):
    """Raw-bacc build (no TileContext): explicit engine streams + ~20
    semaphores instead of Tile's ~65, cutting semaphore init/clear and the
    end-of-kernel barrier butterfly.  Constants (identities, iota ramp,
    block-ones matrix) come packed from the host in two DMAs.  Used for
    m <= 8; larger spans fall back to the Tile build."""
    assert m & (m - 1) == 0 and 1 <= m <= 8
    L = 32 * m
    G = BL * m // 4          # gather groups of 128 rows
    lm = _log2(m)

    c_ones = 0
    c_id48 = 8 * G
    c_idb = c_id48 + 48
    c_ramp = c_idb + 8
    c_base = c_ramp + L
    CW = c_base + 1

    nc = bacc.Bacc("TRN2", target_bir_lowering=False, debug=False,
                   num_devices=NCORES)
    lhs = nc.dram_tensor("lhs", [BL * S, H], F32, kind="ExternalInput").ap()
    posf = nc.dram_tensor("posf", [BL, 2], F32, kind="ExternalInput").ap()
    consts = nc.dram_tensor("consts", [P, CW], F32, kind="ExternalInput").ap()
    pooled_r = nc.dram_tensor("pooled_r", [HC * BL, P], F32,
                              kind="ExternalInput").ap()
    w_r = nc.dram_tensor("w_r", [KC * C, P], F32, kind="ExternalInput").ap()
    bias = nc.dram_tensor("bias", [BL, C], F32, kind="ExternalInput").ap()
    out = nc.dram_tensor("out", [BL, C], F32, kind="ExternalOutput").ap()

    sb = lambda name, shape, dt=F32: nc.alloc_sbuf_tensor(name, shape, dt).ap()
    ps = lambda name, shape: nc.alloc_psum_tensor(name, shape, F32).ap()
    sem = nc.alloc_semaphore

    posf_sb = sb("posf_sb", [BL, 2])
    consts_sb = sb("consts_sb", [P, CW])
    pooled_sb = sb("pooled_sb", [HC * BL, P])
    w_sb = sb("w_sb", [KC * C, P])
    bias_sb = sb("bias_sb", [BL, C])
    rowf = sb("rowf", [BL, L])
    oobf = sb("oobf", [BL, L])
    idxf = sb("idxf", [BL, L])
    idxi = sb("idxi", [BL, L], I32)
    idx_all = sb("idx_all", [P, G], I32)
    lenf = sb("lenf", [BL, 1])
    recip = sb("recip", [BL, 1])
    rows = [sb(f"rows{g}", [P, H]) for g in range(G)]
    asp_sb = sb("asp_sb", [BL, H])
    aspT_sb = sb("aspT_sb", [P, HC * BL])
    pT_sb = sb("pT_sb", [P, HC * BL])
    wT_sb = sb("wT_sb", [P, KC * C])
    out_sb = sb("out_sb", [BL, C])

    sum_a = ps("sum_a", [BL, 512])
    sum_b = ps("sum_b", [BL, H - 512])
    pT_ps = ps("pT_ps", [P, HC * BL])
    wT_ps = ps("wT_ps", [P, KC * C])
    aspT_ps = ps("aspT_ps", [P, HC * BL])
    out_ps = ps("out_ps", [BL, C])

    s_posf, s_consts, s_params = sem("s_posf"), sem("s_consts"), sem("s_params")
    s_ms, s_idx = sem("s_ms"), sem("s_idx")
    s_resh = [sem(f"s_resh{g}") for g in range(G)]
    s_gath = [sem(f"s_gath{g}") for g in range(G)]
    s_ptp, s_pw, s_sum = sem("s_ptp"), sem("s_pw"), sem("s_sum")
    s_scale, s_tp, s_aspT = sem("s_scale"), sem("s_tp"), sem("s_aspT")
    s_fin, s_bias, s_out = sem("s_fin"), sem("s_bias"), sem("s_out")

    A = mybir.AluOpType
    id48 = consts_sb[0:48, c_id48:c_id48 + 48]
    idb = consts_sb[0:8, c_idb:c_idb + 8]
    ramp = consts_sb[0:BL, c_ramp:c_ramp + L]
    base8 = consts_sb[0:BL, c_base:c_base + 1]

    # ---- SCALAR (qActDynamicHW ring): critical-path DMAs -----------
    nc.scalar.dma_start(posf_sb, posf[:, :]).then_inc(s_posf, 16)
    nc.scalar.dma_start(consts_sb, consts[:, :]).then_inc(s_consts, 16)
    # redistribute idx [BL, L] -> [128, 1] per gather group (the indirect
    # DMA reads one offset per offset-tile partition)
    cols = max(1, 4 // m)
    nc.scalar.wait_ge(s_idx, 1)
    for g in range(G):
        s_lo = (4 * g) // m
        j0 = ((4 * g) % m) * 32
        src = (idxi[s_lo:s_lo + cols, j0:j0 + 128 // cols] if cols > 1
               else idxi[s_lo:s_lo + 1, j0:j0 + 128])
        nc.scalar.dma_start(out=idx_all[:, g:g + 1], in_=src).then_inc(
            s_resh[g], 16)

    # ---- SYNC (qSPDynamicHW ring): param loads, output DMA ---------
    nc.sync.dma_start(pooled_sb, pooled_r[:, :]).then_inc(s_params, 16)
    nc.sync.dma_start(w_sb, w_r[:, :]).then_inc(s_params, 16)
    nc.sync.dma_start(bias_sb, bias[:, :]).then_inc(s_params, 16)
    nc.sync.wait_ge(s_bias, 1)
    nc.sync.dma_start(out[:, :], out_sb).then_inc(s_out, 16)
    nc.sync.wait_ge(s_out, 16)

    # ---- DVE: index math, scaling, PSUM evacuation -----------------
    for g in range(G):
        nc.vector.memset(rows[g], 0.0).then_inc(s_ms, 1)
    nc.vector.wait_ge(s_posf, 16)
    nc.vector.wait_ge(s_consts, 16)
    # same-engine RAW chains need pipeline drains in raw mode
    nc.vector.tensor_tensor(out=lenf, in0=posf_sb[:, 1:2],
                            in1=posf_sb[:, 0:1], op=A.subtract)
    nc.vector.tensor_scalar(rowf, ramp, posf_sb[:, 0:1], None, A.add)
    nc.vector.drain()
    nc.vector.reciprocal(recip, lenf)
    nc.vector.tensor_scalar(oobf, rowf, posf_sb[:, 1:2], float(OOB),
                            A.is_ge, A.mult)
    nc.vector.drain()
    nc.vector.scalar_tensor_tensor(idxf, rowf, base8, oobf, A.add, A.add)
    nc.vector.drain()
    nc.vector.tensor_copy(idxi, idxf).then_inc(s_idx, 1)
    nc.vector.wait_ge(s_ptp, 1)
    nc.vector.tensor_copy(pT_sb, pT_ps)
    nc.vector.tensor_copy(wT_sb, wT_ps).then_inc(s_pw, 1)
    nc.vector.wait_ge(s_sum, 1)
    nc.vector.drain()
    nc.vector.tensor_scalar(asp_sb[:, :512], sum_a, recip[:, 0:1], None,
                            A.mult)
    nc.vector.tensor_scalar(asp_sb[:, 512:], sum_b, recip[:, 0:1], None,
                            A.mult).then_inc(s_scale, 1)
    nc.vector.wait_ge(s_tp, 1)
    nc.vector.tensor_copy(aspT_sb, aspT_ps).then_inc(s_aspT, 1)
    nc.vector.wait_ge(s_fin, 1)
    nc.vector.wait_ge(s_params, 48)
    nc.vector.tensor_add(out_sb, out_ps, bias_sb).then_inc(s_bias, 1)

    # ---- GPSIMD: gathers -------------------------------------------
    nc.gpsimd.wait_ge(s_ms, G)
    for g in range(G):
        nc.gpsimd.wait_ge(s_resh[g], 16)
        nc.gpsimd.indirect_dma_start(
            out=rows[g], out_offset=None, in_=lhs[:, :],
            in_offset=IndirectOffsetOnAxis(ap=idx_all[:, g:g + 1], axis=0),
            bounds_check=BL * S - 1, oob_is_err=False,
        ).then_inc(s_gath[g], 16)

    # ---- PE: transposes + sums + final GEMM ------------------------
    nc.tensor.wait_ge(s_params, 48)
    nc.tensor.wait_ge(s_consts, 16)
    nc.tensor.transpose(pT_ps, pooled_sb, id48)
    nc.tensor.transpose(wT_ps, w_sb, id48[:36, :36]).then_inc(s_ptp, 1)
    for g in range(G):
        nc.tensor.wait_ge(s_gath[g], 16)
        first, last = g == 0, g == G - 1
        nc.tensor.matmul(out=sum_a,
                         lhsT=consts_sb[:, 8 * g:8 * g + BL],
                         rhs=rows[g][:, :512], start=first, stop=last)
        mm = nc.tensor.matmul(out=sum_b,
                              lhsT=consts_sb[:, 8 * g:8 * g + BL],
                              rhs=rows[g][:, 512:], start=first, stop=last)
        if last:
            mm.then_inc(s_sum, 1)
    nc.tensor.wait_ge(s_scale, 1)
    for c in range(HC):
        mm = nc.tensor.transpose(aspT_ps[:, c * BL:(c + 1) * BL],
                                 asp_sb[:, c * P:(c + 1) * P], idb)
        if c == HC - 1:
            mm.then_inc(s_tp, 1)
    nc.tensor.wait_ge(s_aspT, 1)
    nc.tensor.wait_ge(s_pw, 1)
    for c in range(KC):
        featT = (pT_sb[:, c * BL:(c + 1) * BL] if c < HC
                 else aspT_sb[:, (c - HC) * BL:(c - HC + 1) * BL])
        mm = nc.tensor.matmul(out=out_ps, lhsT=featT,
                              rhs=wT_sb[:, c * C:(c + 1) * C],
                              start=(c == 0), stop=(c == KC - 1))
    mm.then_inc(s_fin, 1)

    nc.compile()
    return nc


_CACHE: dict[int, object] = {}


def _get(m: int):
    if m not in _CACHE:
        _CACHE[m] = build_raw(m) if m <= 8 else build(m)
    return _CACHE[m]


def kernel(last_hidden_state, pooled_output, position_indices, W, b):
    last_hidden_state = np.ascontiguousarray(last_hidden_state,
                                             dtype=np.float32)
    pooled_output = np.ascontiguousarray(pooled_output, dtype=np.float32)
    position_indices = np.ascontiguousarray(position_indices, dtype=np.int32)
    W = np.ascontiguousarray(W, dtype=np.float32)
    b = np.ascontiguousarray(b, dtype=np.float32)

    lens = position_indices[:, 1] - position_indices[:, 0]
    maxlen = max(1, int(lens.max()))
    m = 1
    while 32 * m < maxlen:
        m *= 2
    nc = _get(m)
    in_maps = _make_in_maps(m, last_hidden_state, pooled_output,
                            position_indices, W, b)
    res = run_bass_kernel_spmd(nc, in_maps, core_ids=list(range(NCORES)),
                               **RUN_KWARGS)
    global LAST_RESULT
    LAST_RESULT = res
    return np.concatenate([res.results[c]["out"] for c in range(NCORES)],
                          axis=0)


def _make_in_maps(m, last_hidden_state, pooled_output, position_indices,
                  W, b):
    w_r = np.ascontiguousarray(
        W.reshape(C, KC, P).transpose(1, 0, 2).reshape(KC * C, P))
    bias_t = np.ascontiguousarray(np.broadcast_to(b[None, :], (BL, C)))
    in_maps = []
    if m <= 8:
        L = 32 * m
        G = BL * m // 4
        lm = _log2(m)
        CW = 8 * G + 57 + L
        consts = np.zeros((P, CW), np.float32)
        p = np.arange(P)
        for g in range(G):
            s_of_p = (4 * g + p // 32) >> lm
            consts[p, 8 * g + s_of_p] = 1.0
        consts[0:48, 8 * G:8 * G + 48] = np.eye(48, dtype=np.float32)
        consts[0:8, 8 * G + 48:8 * G + 56] = np.eye(8, dtype=np.float32)
        consts[0:BL, 8 * G + 56:8 * G + 56 + L] = np.arange(L, dtype=np.float32)
        consts[0:BL, 8 * G + 56 + L] = np.arange(BL, dtype=np.float32) * S
        for cid in range(NCORES):
            sl = slice(cid * BL, (cid + 1) * BL)
            in_maps.append({
                "lhs": last_hidden_state[sl].reshape(BL * S, H),
                "posf": position_indices[sl].astype(np.float32),
                "consts": consts,
                "pooled_r": np.ascontiguousarray(
                    pooled_output[sl].reshape(BL, HC, P).transpose(1, 0, 2)
                    .reshape(HC * BL, P)),
                "w_r": w_r,
                "bias": bias_t,
            })
    else:
        for cid in range(NCORES):
            sl = slice(cid * BL, (cid + 1) * BL)
            in_maps.append({
                "lhs": last_hidden_state[sl].reshape(BL * S, H),
                "pooled_r": np.ascontiguousarray(
                    pooled_output[sl].reshape(BL, HC, P).transpose(1, 0, 2)
                    .reshape(HC * BL, P)),
                "pos": position_indices[sl],
                "w_r": w_r,
                "bias": bias_t,
            })
    return in_maps


# test/bench hooks (harness just calls kernel(); these stay default)
RUN_KWARGS: dict = {}
LAST_RESULT = None


# revision 54
# speedup vs baseline: 1.1185x; 1.1185x over previous
"""Trainium2 Bass kernel: aspect-level sentiment classification head.

  aspect[b] = mean(last_hidden_state[b, start_b:end_b, :])   (ragged spans)
  out = concat([pooled, aspect], -1) @ W.T + b

Strategy: data-parallel over batch (8 samples per core, 8 cores).  The key
observation is that only the span rows of last_hidden_state are ever needed,
so each core *gathers* just those rows from DRAM with an indirect DMA whose
row indices are computed on-device from position_indices.  Spans are padded
to L = 32*m rows (m = power of two chosen from the max span length at call
time); rows past the span end are masked to zero.  The per-sample 1/len is
folded into the mask so a single PE matmul per 128-column chunk produces the
*transposed* aspect features directly, which then feed an accumulated
12-chunk GEMM against host-pre-transposed W.
"""

import os
import sys

if "/opt/trn_rl_repo" not in sys.path:
    sys.path.insert(0, "/opt/trn_rl_repo")

import numpy as np

import concourse.bass as bass
import concourse.tile as tile
from concourse import bacc, mybir
from concourse.bass import IndirectOffsetOnAxis
from concourse.bass_utils import run_bass_kernel_spmd

F32 = mybir.dt.float32
I32 = mybir.dt.int32

B, S, H, C = 64, 4096, 768, 3
NCORES = 8
BL = B // NCORES          # samples per core
P = 128
HC = H // P               # 6 hidden chunks of 128
KC = 2 * H // P           # 12 contraction chunks in the final GEMM


def _log2(x: int) -> int:
    l = x.bit_length() - 1
    assert 1 << l == x
    return l


def build(m: int):
    """Build + compile the per-core SPMD program for spans up to 32*m rows."""
    assert m & (m - 1) == 0 and 1 <= m <= S // 32
    nblk = BL * m            # 32-row blocks per core
    G = nblk // 4            # gather groups of 128 rows
    cols = max(1, 4 // m)    # samples covered by one group
    gps = max(1, m // 4)     # groups per sample
    lm = _log2(m)

    nc = bacc.Bacc("TRN2", target_bir_lowering=False, debug=False,
                   num_devices=NCORES)
    lhs = nc.dram_tensor("lhs", [BL * S, H], F32, kind="ExternalInput").ap()
    pooled_r = nc.dram_tensor("pooled_r", [HC * BL, P], F32,
                              kind="ExternalInput").ap()
    w_r = nc.dram_tensor("w_r", [KC * C, P], F32, kind="ExternalInput").ap()
    pos = nc.dram_tensor("pos", [BL, 2], I32, kind="ExternalInput").ap()
    bias = nc.dram_tensor("bias", [BL, C], F32, kind="ExternalInput").ap()
    out = nc.dram_tensor("out", [BL, C], F32, kind="ExternalOutput").ap()

    with tile.TileContext(nc) as tc:
        with (
            tc.tile_pool(name="const", bufs=1) as cp,
            tc.tile_pool(name="work", bufs=4) as wp,
            tc.tile_pool(name="rows", bufs=4) as rp,
            tc.tile_pool(name="pmisc", bufs=1, space="PSUM") as pm,
            tc.tile_pool(name="pbc", bufs=2, space="PSUM") as pb,
            tc.tile_pool(name="pasp", bufs=1, space="PSUM") as pa,
        ):
            # ---- constants / params -------------------------------------
            id48 = cp.tile([HC * BL, HC * BL], F32, tag="id48")
            from concourse.masks import make_identity
            make_identity(nc, id48[:])

            pos_i = cp.tile([BL, 2], I32, tag="pos_i")
            nc.sync.dma_start(pos_i[:], pos[:, :])
            pos_f = cp.tile([BL, 2], F32, tag="pos_f")
            nc.vector.tensor_copy(pos_f[:], pos_i[:])

            pooled_sb = cp.tile([HC * BL, P], F32, tag="pooled_sb")
            nc.sync.dma_start(pooled_sb[:], pooled_r[:, :])
            w_sb = cp.tile([KC * C, P], F32, tag="w_sb")
            nc.sync.dma_start(w_sb[:], w_r[:, :])
            bias_sb = cp.tile([BL, C], F32, tag="bias_sb")
            nc.sync.dma_start(bias_sb[:], bias[:, :])

            # transpose pooled_r -> pT [128, 48] (pT[h, c*8+b] = pooled[b, c*128+h])
            pT_ps = pm.tile([P, HC * BL], F32, tag="pmisc", name="pT_ps")
            nc.tensor.transpose(pT_ps[:], pooled_sb[:], id48[:])
            pT = cp.tile([P, HC * BL], F32, tag="pT")
            nc.vector.tensor_copy(pT[:], pT_ps[:])

            # transpose w_r -> wT [128, 36] (wT[h, c*3+j] = W[j, c*128+h])
            wT_ps = pm.tile([P, KC * C], F32, tag="pmisc", name="wT_ps")
            nc.tensor.transpose(wT_ps[:], w_sb[:], id48[: KC * C, : KC * C])
            wT = cp.tile([P, KC * C], F32, tag="wT")
            nc.vector.tensor_copy(wT[:], wT_ps[:])

            # per-partition index helpers (p = partition id, u = p >> 5)
            iota_p = cp.tile([P, 1], I32, tag="iota_p")
            nc.gpsimd.iota(iota_p[:], pattern=[[1, 1]], base=0,
                           channel_multiplier=1)
            u_i = cp.tile([P, 1], I32, tag="u_i")
            nc.vector.tensor_scalar(u_i[:], iota_p[:], 5, None,
                                    mybir.AluOpType.arith_shift_right)
            pm32_i = cp.tile([P, 1], I32, tag="pm32_i")
            nc.vector.tensor_scalar(pm32_i[:], iota_p[:], 31, None,
                                    mybir.AluOpType.bitwise_and)
            pm32_f = cp.tile([P, 1], F32, tag="pm32_f")
            nc.vector.tensor_copy(pm32_f[:], pm32_i[:])
            u_f = cp.tile([P, 1], F32, tag="u_f")
            nc.vector.tensor_copy(u_f[:], u_i[:])

            # psum accumulators for the span sums [BL, H] (two banks);
            # every gather group's matmul writes all BL rows (zeros in the
            # mask columns of other samples) and accumulates
            sum_a = pa.tile([BL, 512], F32, tag="sum_a")
            sum_b = pa.tile([BL, H - 512], F32, tag="sum_b")

            # ---- gather groups ------------------------------------------
            for g in range(G):
                # broadcast (start, end) of each partition's sample via PE:
                # ind[s, p] = 1 iff s == (4g + p//32) >> lm
                ind = wp.tile([BL, P], F32, tag="ind")
                nc.gpsimd.memset(ind[:], 1.0)
                nc.gpsimd.affine_select(
                    out=ind[:], in_=ind[:], compare_op=mybir.AluOpType.is_ge,
                    fill=0.0, base=128 * g, channel_multiplier=-32 * m,
                    pattern=[[1, P]])
                # keep where p - 32m*s + 128g <= 32m-1, negated for is_ge
                nc.gpsimd.affine_select(
                    out=ind[:], in_=ind[:], compare_op=mybir.AluOpType.is_ge,
                    fill=0.0, base=(32 * m - 1) - 128 * g,
                    channel_multiplier=32 * m, pattern=[[-1, P]])
                bc_ps = pb.tile([P, 2], F32, tag="bc")
                nc.tensor.matmul(out=bc_ps[:], lhsT=ind[:], rhs=pos_f[:],
                                 start=True, stop=True)
                bc = wp.tile([P, 2], F32, tag="bcs")
                nc.vector.tensor_copy(bc[:], bc_ps[:])
                st_f = bc[:, 0:1]
                en_f = bc[:, 1:2]

                # row-within-span and sample base offset for this group
                if m == 1:
                    jrow_f = pm32_f[:]
                    s4096_f = wp.tile([P, 1], F32, tag="s4096")
                    # (u + 4g) * 4096
                    nc.vector.tensor_scalar(
                        s4096_f[:], u_f[:], float(4 * g), 4096.0,
                        mybir.AluOpType.add, mybir.AluOpType.mult)
                else:
                    k_i = wp.tile([P, 1], I32, tag="k_i")
                    nc.vector.tensor_scalar(k_i[:], u_i[:], 4 * g, None,
                                            mybir.AluOpType.add)
                    q32_i = wp.tile([P, 1], I32, tag="q32")
                    nc.vector.tensor_scalar(
                        q32_i[:], k_i[:], m - 1, 32,
                        mybir.AluOpType.bitwise_and, mybir.AluOpType.mult)
                    jr_i = wp.tile([P, 1], I32, tag="jr_i")
                    nc.vector.tensor_add(jr_i[:], q32_i[:], pm32_i[:])
                    jrow_ft = wp.tile([P, 1], F32, tag="jrow_f")
                    nc.vector.tensor_copy(jrow_ft[:], jr_i[:])
                    jrow_f = jrow_ft[:]
                    s4_i = wp.tile([P, 1], I32, tag="s4_i")
                    nc.vector.tensor_scalar(
                        s4_i[:], k_i[:], lm, 4096,
                        mybir.AluOpType.arith_shift_right,
                        mybir.AluOpType.mult)
                    s4096_f = wp.tile([P, 1], F32, tag="s4096")
                    nc.vector.tensor_copy(s4096_f[:], s4_i[:])

                # gather row index = min(start + jrow, S-1) + 4096*s
                row_f = wp.tile([P, 1], F32, tag="row_f")
                nc.vector.tensor_add(row_f[:], st_f, jrow_f)
                idx_f = wp.tile([P, 1], F32, tag="idx_f")
                nc.vector.tensor_scalar(
                    idx_f[:], row_f[:], float(S - 1), s4096_f[:, 0:1],
                    mybir.AluOpType.min, mybir.AluOpType.add)
                idx_i = wp.tile([P, 1], I32, tag="idx_i")
                nc.vector.tensor_copy(idx_i[:], idx_f[:])

                # mask = (jrow < len) / len  (len==0 -> NaN, matches 0/0 ref)
                len_f = wp.tile([P, 1], F32, tag="len_f")
                nc.vector.tensor_sub(len_f[:], en_f, st_f)
                recip = wp.tile([P, 1], F32, tag="recip")
                nc.vector.reciprocal(recip[:], len_f[:])
                inm = wp.tile([P, 1], F32, tag="inm")
                nc.vector.tensor_tensor(out=inm[:], in0=jrow_f, in1=len_f[:],
                                        op=mybir.AluOpType.is_lt)
                inm_s = wp.tile([P, 1], F32, tag="inm_s")
                nc.vector.tensor_mul(inm_s[:], inm[:], recip[:])

                # full-width mask [128, BL]: column s holds (jrow<len)/len for
                # rows of sample s in this group, 0 elsewhere
                mk = wp.tile([P, BL], F32, tag="mk")
                nc.vector.tensor_copy(mk[:], inm_s[:, 0:1].to_broadcast(
                    [P, BL]))
                # keep where p - 32m*s + 128g in [0, 32m)
                nc.gpsimd.affine_select(
                    out=mk[:], in_=mk[:],
                    compare_op=mybir.AluOpType.is_ge, fill=0.0, base=128 * g,
                    channel_multiplier=1, pattern=[[-32 * m, BL]])
                nc.gpsimd.affine_select(
                    out=mk[:], in_=mk[:],
                    compare_op=mybir.AluOpType.is_ge, fill=0.0,
                    base=32 * m - 1 - 128 * g, channel_multiplier=-1,
                    pattern=[[32 * m, BL]])

                rows_t = rp.tile([P, H], F32, tag="rows")
                nc.gpsimd.indirect_dma_start(
                    out=rows_t[:], out_offset=None, in_=lhs[:, :],
                    in_offset=IndirectOffsetOnAxis(ap=idx_i[:, 0:1], axis=0))

                # span means accumulate: sum[s, :] += mask[:, s].T @ rows
                first, last = g == 0, g == G - 1
                nc.tensor.matmul(out=sum_a[:], lhsT=mk[:],
                                 rhs=rows_t[:, :512], start=first, stop=last)
                nc.tensor.matmul(out=sum_b[:], lhsT=mk[:],
                                 rhs=rows_t[:, 512:], start=first, stop=last)

            # ---- transpose means into GEMM layout ------------------------
            asp_sb = cp.tile([BL, H], F32, tag="asp_sb")
            nc.vector.tensor_copy(asp_sb[:, :512], sum_a[:])
            nc.vector.tensor_copy(asp_sb[:, 512:], sum_b[:])
            aspT_ps = pm.tile([P, HC * BL], F32, tag="pmisc", name="aspT_ps")
            for c in range(HC):
                nc.tensor.transpose(aspT_ps[:, c * BL:(c + 1) * BL],
                                    asp_sb[:, c * P:(c + 1) * P],
                                    id48[:BL, :BL])

            # ---- final GEMM: out[b, j] = sum_k featT[k, b] * wT[k, j] ----
            aspT_sb = cp.tile([P, HC * BL], F32, tag="aspT_sb")
            nc.vector.tensor_copy(aspT_sb[:], aspT_ps[:])

            out_ps = pm.tile([BL, C], F32, tag="pmisc", name="out_ps")
            for c in range(KC):
                featT = (pT[:, (c * BL):(c + 1) * BL] if c < HC
                         else aspT_sb[:, (c - HC) * BL:(c - HC + 1) * BL])
                nc.tensor.matmul(out=out_ps[:], lhsT=featT,
                                 rhs=wT[:, c * C:(c + 1) * C],
                                 start=(c == 0), stop=(c == KC - 1))

            out_sb = cp.tile([BL, C], F32, tag="out_sb")
            nc.vector.tensor_add(out_sb[:], out_ps[:], bias_sb[:])
            nc.sync.dma_start(out[:, :], out_sb[:])

    nc.compile()
    return nc


_CACHE: dict[int, object] = {}


def _get(m: int):
    if m not in _CACHE:
        _CACHE[m] = build(m)
    return _CACHE[m]


def kernel(last_hidden_state, pooled_output, position_indices, W, b):
    last_hidden_state = np.ascontiguousarray(last_hidden_state,
                                             dtype=np.float32)
    pooled_output = np.ascontiguousarray(pooled_output, dtype=np.float32)
    position_indices = np.ascontiguousarray(position_indices, dtype=np.int32)
    W = np.ascontiguousarray(W, dtype=np.float32)
    b = np.ascontiguousarray(b, dtype=np.float32)

    lens = position_indices[:, 1] - position_indices[:, 0]
    maxlen = max(1, int(lens.max()))
    m = 1
    while 32 * m < maxlen:
        m *= 2
    nc = _get(m)
    in_maps = _make_in_maps(m, last_hidden_state, pooled_output,
                            position_indices, W, b)
    res = run_bass_kernel_spmd(nc, in_maps, core_ids=list(range(NCORES)),
                               **RUN_KWARGS)
    global LAST_RESULT
    LAST_RESULT = res
    return np.concatenate([res.results[c]["out"] for c in range(NCORES)],
                          axis=0)


def _make_in_maps(m, last_hidden_state, pooled_output, position_indices,
                  W, b):
    w_r = np.ascontiguousarray(
        W.reshape(C, KC, P).transpose(1, 0, 2).reshape(KC * C, P))
    bias_t = np.ascontiguousarray(np.broadcast_to(b[None, :], (BL, C)))
    in_maps = []
    for cid in range(NCORES):
        sl = slice(cid * BL, (cid + 1) * BL)
        in_maps.append({
            "lhs": last_hidden_state[sl].reshape(BL * S, H),
            "pooled_r": np.ascontiguousarray(
                pooled_output[sl].reshape(BL, HC, P).transpose(1, 0, 2)
                .reshape(HC * BL, P)),
            "pos": position_indices[sl],
            "w_r": w_r,
            "bias": bias_t,
        })
    return in_maps


# test/bench hooks (harness just calls kernel(); these stay default)
RUN_KWARGS: dict = {}
LAST_RESULT = None


# revision 55
# speedup vs baseline: 1.3201x; 1.1803x over previous
"""Trainium2 Bass kernel: aspect-level sentiment classification head.

  aspect[b] = mean(last_hidden_state[b, start_b:end_b, :])   (ragged spans)
  out = concat([pooled, aspect], -1) @ W.T + b

Strategy: data-parallel over batch (8 samples per core, 8 cores).  The key
observation is that only the span rows of last_hidden_state are ever needed,
so each core *gathers* just those rows from DRAM with an indirect DMA whose
row indices are computed on-device from position_indices.  Spans are padded
to L = 32*m rows (m = power of two chosen from the max span length at call
time); rows past the span end are masked to zero.  The per-sample 1/len is
folded into the mask so a single PE matmul per 128-column chunk produces the
*transposed* aspect features directly, which then feed an accumulated
12-chunk GEMM against host-pre-transposed W.
"""

import os
import sys

if "/opt/trn_rl_repo" not in sys.path:
    sys.path.insert(0, "/opt/trn_rl_repo")

import numpy as np

import concourse.bass as bass
import concourse.tile as tile
from concourse import bacc, mybir
from concourse.bass import IndirectOffsetOnAxis
from concourse.bass_utils import run_bass_kernel_spmd

F32 = mybir.dt.float32
I32 = mybir.dt.int32

B, S, H, C = 64, 4096, 768, 3
NCORES = 8
BL = B // NCORES          # samples per core
P = 128
HC = H // P               # 6 hidden chunks of 128
KC = 2 * H // P           # 12 contraction chunks in the final GEMM


def _log2(x: int) -> int:
    l = x.bit_length() - 1
    assert 1 << l == x
    return l


def build(m: int):
    """Build + compile the per-core SPMD program for spans up to 32*m rows."""
    assert m & (m - 1) == 0 and 1 <= m <= S // 32
    nblk = BL * m            # 32-row blocks per core
    G = nblk // 4            # gather groups of 128 rows
    cols = max(1, 4 // m)    # samples covered by one group
    gps = max(1, m // 4)     # groups per sample
    lm = _log2(m)

    nc = bacc.Bacc("TRN2", target_bir_lowering=False, debug=False,
                   num_devices=NCORES)
    lhs = nc.dram_tensor("lhs", [BL * S, H], F32, kind="ExternalInput").ap()
    pooled_r = nc.dram_tensor("pooled_r", [HC * BL, P], F32,
                              kind="ExternalInput").ap()
    w_r = nc.dram_tensor("w_r", [KC * C, P], F32, kind="ExternalInput").ap()
    pos = nc.dram_tensor("pos", [BL, 2], I32, kind="ExternalInput").ap()
    bias = nc.dram_tensor("bias", [BL, C], F32, kind="ExternalInput").ap()
    out = nc.dram_tensor("out", [BL, C], F32, kind="ExternalOutput").ap()

    with tile.TileContext(nc) as tc:
        packed = m <= 4  # one PSUM bank for all 6 aspect chunks vs 6 banks
        with (
            tc.tile_pool(name="const", bufs=1) as cp,
            tc.tile_pool(name="work", bufs=4) as wp,
            tc.tile_pool(name="rows", bufs=4) as rp,
            tc.tile_pool(name="pmisc", bufs=1, space="PSUM") as pm,
            tc.tile_pool(name="pbc", bufs=2 if packed else 1,
                         space="PSUM") as pb,
            tc.tile_pool(name="pasp", bufs=1, space="PSUM") as pa,
        ):
            # ---- constants / params -------------------------------------
            id48 = cp.tile([HC * BL, HC * BL], F32, tag="id48")
            from concourse.masks import make_identity
            make_identity(nc, id48[:])

            pos_i = cp.tile([BL, 2], I32, tag="pos_i")
            nc.sync.dma_start(pos_i[:], pos[:, :])
            pos_f = cp.tile([BL, 2], F32, tag="pos_f")
            nc.vector.tensor_copy(pos_f[:], pos_i[:])

            pooled_sb = cp.tile([HC * BL, P], F32, tag="pooled_sb")
            nc.sync.dma_start(pooled_sb[:], pooled_r[:, :])
            w_sb = cp.tile([KC * C, P], F32, tag="w_sb")
            nc.sync.dma_start(w_sb[:], w_r[:, :])
            bias_sb = cp.tile([BL, C], F32, tag="bias_sb")
            nc.sync.dma_start(bias_sb[:], bias[:, :])

            # transpose pooled_r -> pT [128, 48] (pT[h, c*8+b] = pooled[b, c*128+h])
            pT_ps = pm.tile([P, HC * BL], F32, tag="pmisc", name="pT_ps")
            nc.tensor.transpose(pT_ps[:], pooled_sb[:], id48[:])
            pT = cp.tile([P, HC * BL], F32, tag="pT")
            nc.vector.tensor_copy(pT[:], pT_ps[:])

            # transpose w_r -> wT [128, 36] (wT[h, c*3+j] = W[j, c*128+h])
            wT_ps = pm.tile([P, KC * C], F32, tag="pmisc", name="wT_ps")
            nc.tensor.transpose(wT_ps[:], w_sb[:], id48[: KC * C, : KC * C])
            wT = cp.tile([P, KC * C], F32, tag="wT")
            nc.vector.tensor_copy(wT[:], wT_ps[:])

            # per-partition index helpers (p = partition id, u = p >> 5)
            iota_p = cp.tile([P, 1], I32, tag="iota_p")
            nc.gpsimd.iota(iota_p[:], pattern=[[1, 1]], base=0,
                           channel_multiplier=1)
            u_i = cp.tile([P, 1], I32, tag="u_i")
            nc.vector.tensor_scalar(u_i[:], iota_p[:], 5, None,
                                    mybir.AluOpType.arith_shift_right)
            pm32_i = cp.tile([P, 1], I32, tag="pm32_i")
            nc.vector.tensor_scalar(pm32_i[:], iota_p[:], 31, None,
                                    mybir.AluOpType.bitwise_and)
            pm32_f = cp.tile([P, 1], F32, tag="pm32_f")
            nc.vector.tensor_copy(pm32_f[:], pm32_i[:])
            u_f = cp.tile([P, 1], F32, tag="u_f")
            nc.vector.tensor_copy(u_f[:], u_i[:])

            # psum accumulators for transposed aspect features; for m >= 8
            # accumulation groups stay open across gather groups, so each
            # hidden chunk needs its own bank
            if packed:
                aspT_all = pa.tile([P, HC * BL], F32, tag="aspT")
                aspT_ps = [aspT_all[:, c * BL:(c + 1) * BL]
                           for c in range(HC)]
            else:
                aspT_ps = [pa.tile([P, BL], F32, tag=f"aspT{c}",
                                   name=f"aspT{c}")[:] for c in range(HC)]

            # ---- gather groups ------------------------------------------
            for g in range(G):
                # broadcast (start, end) of each partition's sample via PE:
                # ind[s, p] = 1 iff s == (4g + p//32) >> lm
                ind = wp.tile([BL, P], F32, tag="ind")
                nc.gpsimd.memset(ind[:], 1.0)
                nc.gpsimd.affine_select(
                    out=ind[:], in_=ind[:], compare_op=mybir.AluOpType.is_ge,
                    fill=0.0, base=128 * g, channel_multiplier=-32 * m,
                    pattern=[[1, P]])
                # keep where p - 32m*s + 128g <= 32m-1, negated for is_ge
                nc.gpsimd.affine_select(
                    out=ind[:], in_=ind[:], compare_op=mybir.AluOpType.is_ge,
                    fill=0.0, base=(32 * m - 1) - 128 * g,
                    channel_multiplier=32 * m, pattern=[[-1, P]])
                bc_ps = pb.tile([P, 2], F32, tag="bc")
                nc.tensor.matmul(out=bc_ps[:], lhsT=ind[:], rhs=pos_f[:],
                                 start=True, stop=True)
                bc = wp.tile([P, 2], F32, tag="bcs")
                nc.vector.tensor_copy(bc[:], bc_ps[:])
                st_f = bc[:, 0:1]
                en_f = bc[:, 1:2]

                # row-within-span and sample base offset for this group
                if m == 1:
                    jrow_f = pm32_f[:]
                    s4096_f = wp.tile([P, 1], F32, tag="s4096")
                    # (u + 4g) * 4096
                    nc.vector.tensor_scalar(
                        s4096_f[:], u_f[:], float(4 * g), 4096.0,
                        mybir.AluOpType.add, mybir.AluOpType.mult)
                else:
                    k_i = wp.tile([P, 1], I32, tag="k_i")
                    nc.vector.tensor_scalar(k_i[:], u_i[:], 4 * g, None,
                                            mybir.AluOpType.add)
                    q32_i = wp.tile([P, 1], I32, tag="q32")
                    nc.vector.tensor_scalar(
                        q32_i[:], k_i[:], m - 1, 32,
                        mybir.AluOpType.bitwise_and, mybir.AluOpType.mult)
                    jr_i = wp.tile([P, 1], I32, tag="jr_i")
                    nc.vector.tensor_add(jr_i[:], q32_i[:], pm32_i[:])
                    jrow_ft = wp.tile([P, 1], F32, tag="jrow_f")
                    nc.vector.tensor_copy(jrow_ft[:], jr_i[:])
                    jrow_f = jrow_ft[:]
                    s4_i = wp.tile([P, 1], I32, tag="s4_i")
                    nc.vector.tensor_scalar(
                        s4_i[:], k_i[:], lm, 4096,
                        mybir.AluOpType.arith_shift_right,
                        mybir.AluOpType.mult)
                    s4096_f = wp.tile([P, 1], F32, tag="s4096")
                    nc.vector.tensor_copy(s4096_f[:], s4_i[:])

                # gather row index = min(start + jrow, S-1) + 4096*s
                row_f = wp.tile([P, 1], F32, tag="row_f")
                nc.vector.tensor_add(row_f[:], st_f, jrow_f)
                idx_f = wp.tile([P, 1], F32, tag="idx_f")
                nc.vector.tensor_scalar(
                    idx_f[:], row_f[:], float(S - 1), s4096_f[:, 0:1],
                    mybir.AluOpType.min, mybir.AluOpType.add)
                idx_i = wp.tile([P, 1], I32, tag="idx_i")
                nc.vector.tensor_copy(idx_i[:], idx_f[:])

                # mask = (jrow < len) / len  (len==0 -> NaN, matches 0/0 ref)
                len_f = wp.tile([P, 1], F32, tag="len_f")
                nc.vector.tensor_sub(len_f[:], en_f, st_f)
                recip = wp.tile([P, 1], F32, tag="recip")
                nc.vector.reciprocal(recip[:], len_f[:])
                inm = wp.tile([P, 1], F32, tag="inm")
                nc.vector.tensor_tensor(out=inm[:], in0=jrow_f, in1=len_f[:],
                                        op=mybir.AluOpType.is_lt)
                inm_s = wp.tile([P, 1], F32, tag="inm_s")
                nc.vector.tensor_mul(inm_s[:], inm[:], recip[:])

                if cols == 1:
                    maskg = inm_s[:]
                else:
                    mk = wp.tile([P, cols], F32, tag="mk")
                    nc.vector.tensor_copy(mk[:], inm_s[:, 0:1].to_broadcast(
                        [P, cols]))
                    nc.gpsimd.affine_select(
                        out=mk[:], in_=mk[:],
                        compare_op=mybir.AluOpType.is_ge, fill=0.0, base=0,
                        channel_multiplier=1, pattern=[[-32 * m, cols]])
                    # keep where p - 32m*j <= 32m-1, negated for is_ge
                    nc.gpsimd.affine_select(
                        out=mk[:], in_=mk[:],
                        compare_op=mybir.AluOpType.is_ge, fill=0.0,
                        base=32 * m - 1, channel_multiplier=-1,
                        pattern=[[32 * m, cols]])
                    maskg = mk[:]

                rows_t = rp.tile([P, H], F32, tag="rows")
                nc.gpsimd.indirect_dma_start(
                    out=rows_t[:], out_offset=None, in_=lhs[:, :],
                    in_offset=IndirectOffsetOnAxis(ap=idx_i[:, 0:1], axis=0))

                # aspT[h, s] += rows[:, chunk].T @ mask
                s_lo = (4 * g) // m
                first = g % gps == 0
                last = g % gps == gps - 1
                for c in range(HC):
                    nc.tensor.matmul(
                        out=aspT_ps[c][:, s_lo:s_lo + cols],
                        lhsT=rows_t[:, c * P:(c + 1) * P], rhs=maskg,
                        start=first, stop=last)

            # ---- final GEMM: out[b, j] = sum_k featT[k, b] * wT[k, j] ----
            aspT_sb = cp.tile([P, HC * BL], F32, tag="aspT_sb")
            if packed:
                nc.vector.tensor_copy(aspT_sb[:], aspT_all[:])
            else:
                for c in range(HC):
                    nc.vector.tensor_copy(aspT_sb[:, c * BL:(c + 1) * BL],
                                          aspT_ps[c])

            out_ps = pm.tile([BL, C], F32, tag="pmisc", name="out_ps")
            for c in range(KC):
                featT = (pT[:, (c * BL):(c + 1) * BL] if c < HC
                         else aspT_sb[:, (c - HC) * BL:(c - HC + 1) * BL])
                nc.tensor.matmul(out=out_ps[:], lhsT=featT,
                                 rhs=wT[:, c * C:(c + 1) * C],
                                 start=(c == 0), stop=(c == KC - 1))

            out_sb = cp.tile([BL, C], F32, tag="out_sb")
            nc.vector.tensor_add(out_sb[:], out_ps[:], bias_sb[:])
            nc.sync.dma_start(out[:, :], out_sb[:])

    nc.compile()
    return nc


_CACHE: dict[int, object] = {}


def _get(m: int):
    if m not in _CACHE:
        _CACHE[m] = build(m)
    return _CACHE[m]


def kernel(last_hidden_state, pooled_output, position_indices, W, b):
    last_hidden_state = np.ascontiguousarray(last_hidden_state,
                                             dtype=np.float32)
    pooled_output = np.ascontiguousarray(pooled_output, dtype=np.float32)
    position_indices = np.ascontiguousarray(position_indices, dtype=np.int32)
    W = np.ascontiguousarray(W, dtype=np.float32)
    b = np.ascontiguousarray(b, dtype=np.float32)

    lens = position_indices[:, 1] - position_indices[:, 0]
    maxlen = max(1, int(lens.max()))
    m = 1
    while 32 * m < maxlen:
        m *= 2
    nc = _get(m)

    w_r = np.ascontiguousarray(
        W.reshape(C, KC, P).transpose(1, 0, 2).reshape(KC * C, P))
    bias_t = np.ascontiguousarray(np.broadcast_to(b[None, :], (BL, C)))
    in_maps = []
    for cid in range(NCORES):
        sl = slice(cid * BL, (cid + 1) * BL)
        in_maps.append({
            "lhs": last_hidden_state[sl].reshape(BL * S, H),
            "pooled_r": np.ascontiguousarray(
                pooled_output[sl].reshape(BL, HC, P).transpose(1, 0, 2)
                .reshape(HC * BL, P)),
            "pos": position_indices[sl],
            "w_r": w_r,
            "bias": bias_t,
        })
    res = run_bass_kernel_spmd(nc, in_maps, core_ids=list(range(NCORES)),
                               **RUN_KWARGS)
    global LAST_RESULT
    LAST_RESULT = res
    return np.concatenate([res.results[c]["out"] for c in range(NCORES)],
                          axis=0)


# test/bench hooks (harness just calls kernel(); these stay default)
RUN_KWARGS: dict = {}
LAST_RESULT = None
